# revision 7
# baseline (speedup 1.0000x reference)
"""Batched einsum bik,bkj->bij on 8 TRN2 NeuronCores.

Full shapes: input_0 [32, 2048, 128] f32, input_1 [32, 128, 2048] f32,
output [32, 2048, 2048] f32. Batch axis is sharded 4-per-core (SPMD, no
communication). Per core: 4 matmuls of (2048x128)@(128x2048).

The contraction dim K=128 maps exactly onto the PE partition dim. B loads
naturally as the moving operand [K, J]; A ([I, K] in DRAM) is transposed
on the PE (identity-matmul transpose, the fp32 path) into lhsT tiles
[K, 128i]. Matmul outputs land in PSUM [128, 512] (one bank), are evicted
by ScalarE/VectorE alternately into [128, 2048] SBUF staging rows, and
DMA'd out as 1 MiB contiguous transfers.
"""

import numpy as np

import concourse.bass as bass
import concourse.mybir as mybir
import concourse.tile as tile
from concourse import bacc
from concourse.bass_utils import run_bass_kernel_spmd
from concourse.masks import make_identity

B, I, K, J = 32, 2048, 128, 2048
N_CORES = 8
BPC = B // N_CORES  # batches per core
P = 128
M_TILES = I // P  # 16
N_TILE = 512  # fp32 moving-operand / PSUM-bank limit
N_TILES = J // N_TILE  # 4

# 'f32' = exact fp32 matmul (4 cyc/row); 'f32r' = single-pass reduced-precision
# fp32 matmul (1 cyc/row at N>=256).
MM_DTYPE = "f32r"


def build_nc():
    nc = bacc.Bacc("TRN2", target_bir_lowering=False)
    a = nc.dram_tensor("a", [BPC, I, K], mybir.dt.float32, kind="ExternalInput")
    b = nc.dram_tensor("b", [BPC, K, J], mybir.dt.float32, kind="ExternalInput")
    c = nc.dram_tensor("c", [BPC, I, J], mybir.dt.float32, kind="ExternalOutput")

    f32 = mybir.dt.float32
    mm_dt = mybir.dt.float32r if MM_DTYPE == "f32r" else f32

    with tile.TileContext(nc) as tc:
        with (
            tc.tile_pool(name="const", bufs=1) as const_pool,
            tc.tile_pool(name="a_nat", bufs=2) as a_nat_pool,
            tc.tile_pool(name="kxm", bufs=BPC) as kxm_pool,
            tc.tile_pool(name="kxn", bufs=BPC) as kxn_pool,
            tc.tile_pool(name="ostage", bufs=3) as out_pool,
            tc.tile_pool(name="psum", bufs=6, space="PSUM") as psum_pool,
            tc.tile_pool(name="tpsum", bufs=2, space="PSUM") as tpsum_pool,
        ):
            identity = const_pool.tile([P, P], f32)
            make_identity(nc, identity)

            # Load all inputs and pre-transpose every A block first so the
            # main matmul stream runs back-to-back on the PE afterwards.
            kxns = []
            kxms = []
            for bi in range(BPC):
                kxn = kxn_pool.tile([P, J], mm_dt, tag="kxn")
                if MM_DTYPE == "f32r":
                    # fp32r operands must be rounded by an engine op; bounce
                    # the DMA through an fp32 staging tile and cast-copy.
                    b_stage = a_nat_pool.tile([P, J], f32, tag="b_stage")
                    nc.sync.dma_start(b_stage[:], b[bi])
                    nc.vector.tensor_copy(kxn[:], b_stage[:])
                else:
                    nc.sync.dma_start(kxn[:], b[bi])
                kxns.append(kxn)

                a_nat = a_nat_pool.tile([P, M_TILES, K], f32, tag="a_nat")
                nc.sync.dma_start(
                    a_nat[:], a[bi].rearrange("(io p) k -> p io k", p=P)
                )
                kxm = kxm_pool.tile([P, M_TILES, P], mm_dt, tag="kxm")
                for m in range(M_TILES):
                    tp = tpsum_pool.tile([P, P], f32, tag="tp")
                    nc.tensor.transpose(tp[:], a_nat[:, m, :], identity)
                    if m % 2 == 0:
                        nc.scalar.copy(kxm[:, m, :], tp[:])
                    else:
                        nc.vector.tensor_copy(kxm[:, m, :], tp[:])
                kxms.append(kxm)

            # 2 m-tiles share one staging tile -> 2 MiB output DMAs.
            M_PAIR = 2
            evict = 0
            for bi in range(BPC):
                kxn = kxns[bi]
                kxm = kxms[bi]
                for mp in range(M_TILES // M_PAIR):
                    ostage = out_pool.tile([P, M_PAIR, J], f32, tag="ostage")
                    for mo in range(M_PAIR):
                        m = mp * M_PAIR + mo
                        for n in range(N_TILES):
                            ps = psum_pool.tile([P, N_TILE], f32, tag="ps")
                            nc.tensor.matmul(
                                ps[:],
                                kxm[:, m, :],
                                kxn[:, n * N_TILE : (n + 1) * N_TILE],
                                start=True,
                                stop=True,
                            )
                            dst = ostage[:, mo, n * N_TILE : (n + 1) * N_TILE]
                            if evict % 2 == 0:
                                nc.scalar.copy(dst, ps[:])
                            else:
                                nc.vector.tensor_copy(dst, ps[:])
                            evict += 1
                    rows = M_PAIR * P
                    nc.sync.dma_start(
                        c[bi, mp * rows : (mp + 1) * rows, :].rearrange(
                            "(mo p) j -> p mo j", p=P
                        ),
                        ostage[:],
                    )

    nc.compile()
    return nc


_NC_CACHE = None


def get_nc():
    global _NC_CACHE
    if _NC_CACHE is None:
        _NC_CACHE = build_nc()
    return _NC_CACHE


def kernel(input_0: np.ndarray, input_1: np.ndarray, _trace=False) -> np.ndarray:
    input_0 = np.ascontiguousarray(np.asarray(input_0, dtype=np.float32))
    input_1 = np.ascontiguousarray(np.asarray(input_1, dtype=np.float32))
    nc = get_nc()
    in_maps = [
        {
            "a": input_0[ci * BPC : (ci + 1) * BPC],
            "b": input_1[ci * BPC : (ci + 1) * BPC],
        }
        for ci in range(N_CORES)
    ]
    res = run_bass_kernel_spmd(nc, in_maps, list(range(N_CORES)), trace=_trace)
    out = np.concatenate([res.results[ci]["c"] for ci in range(N_CORES)], axis=0)
    if _trace:
        kernel.last_result = res
    return out


# revision 13
# speedup vs baseline: 1.1713x; 1.1713x over previous
"""Batched einsum bik,bkj->bij on 8 TRN2 NeuronCores.

Full shapes: input_0 [32, 2048, 128] f32, input_1 [32, 128, 2048] f32,
output [32, 2048, 2048] f32. Batch axis is sharded 4-per-core (SPMD, no
communication). Per core: 4 matmuls of (2048x128)@(128x2048).

The contraction dim K=128 maps exactly onto the PE partition dim. B loads
naturally as the moving operand [K, J]; A ([I, K] in DRAM) is transposed
on the PE (identity-matmul transpose, the fp32 path) into lhsT tiles
[K, 128i]. Matmul outputs land in PSUM [128, 512] (one bank), are evicted
by ScalarE/VectorE alternately into [128, 2048] SBUF staging rows, and
DMA'd out as 1 MiB contiguous transfers.
"""

import numpy as np

import concourse.bass as bass
import concourse.mybir as mybir
import concourse.tile as tile
from concourse import bacc
from concourse.bass_utils import run_bass_kernel_spmd
from concourse.masks import make_identity

B, I, K, J = 32, 2048, 128, 2048
N_CORES = 8
BPC = B // N_CORES  # batches per core
P = 128
M_TILES = I // P  # 16
N_TILE = 512  # fp32 moving-operand / PSUM-bank limit
N_TILES = J // N_TILE  # 4

# 'f32' = exact fp32 matmul (4 cyc/row); 'f32r' = single-pass reduced-precision
# fp32 matmul (1 cyc/row at N>=256).
MM_DTYPE = "f32r"


def build_nc():
    nc = bacc.Bacc("TRN2", target_bir_lowering=False)
    a = nc.dram_tensor("a", [BPC, I, K], mybir.dt.float32, kind="ExternalInput")
    b = nc.dram_tensor("b", [BPC, K, J], mybir.dt.float32, kind="ExternalInput")
    c = nc.dram_tensor("c", [BPC, I, J], mybir.dt.float32, kind="ExternalOutput")

    f32 = mybir.dt.float32
    mm_dt = mybir.dt.float32r if MM_DTYPE == "f32r" else f32

    with tile.TileContext(nc) as tc:
        with (
            tc.tile_pool(name="const", bufs=1) as const_pool,
            tc.tile_pool(name="a_nat", bufs=2) as a_nat_pool,
            tc.tile_pool(name="kxm", bufs=BPC) as kxm_pool,
            tc.tile_pool(name="kxn", bufs=BPC) as kxn_pool,
            tc.tile_pool(name="ostage", bufs=4) as out_pool,
            tc.tile_pool(name="psum", bufs=6, space="PSUM") as psum_pool,
            tc.tile_pool(name="tpsum", bufs=2, space="PSUM") as tpsum_pool,
        ):
            identity = const_pool.tile([P, P], f32)
            make_identity(nc, identity)

            # Load all inputs and pre-transpose every A block first so the
            # main matmul stream runs back-to-back on the PE afterwards.
            kxns = []
            kxms = []
            for bi in range(BPC):
                kxn = kxn_pool.tile([P, J], mm_dt, tag="kxn")
                if MM_DTYPE == "f32r":
                    # fp32r operands must be rounded by an engine op; bounce
                    # the DMA through an fp32 staging tile and cast-copy.
                    b_stage = a_nat_pool.tile([P, J], f32, tag="b_stage")
                    nc.sync.dma_start(b_stage[:], b[bi])
                    nc.vector.tensor_copy(kxn[:], b_stage[:])
                else:
                    nc.sync.dma_start(kxn[:], b[bi])
                kxns.append(kxn)

                a_nat = a_nat_pool.tile([P, M_TILES, K], f32, tag="a_nat")
                nc.sync.dma_start(
                    a_nat[:], a[bi].rearrange("(io p) k -> p io k", p=P)
                )
                kxm = kxm_pool.tile([P, M_TILES, P], mm_dt, tag="kxm")
                for m in range(M_TILES):
                    tp = tpsum_pool.tile([P, P], f32, tag="tp")
                    nc.tensor.transpose(tp[:], a_nat[:, m, :], identity)
                    if m % 2 == 0:
                        nc.scalar.copy(kxm[:, m, :], tp[:])
                    else:
                        nc.vector.tensor_copy(kxm[:, m, :], tp[:])
                kxms.append(kxm)

            # M_PAIR m-tiles share one staging tile per output DMA. 1 MiB
            # transfers (M_PAIR=1) measured faster end-to-end than 2 MiB:
            # bigger DMAs drop engine-busy but coarsen the pipeline.
            M_PAIR = 1
            evict = 0
            for bi in range(BPC):
                kxn = kxns[bi]
                kxm = kxms[bi]
                for mp in range(M_TILES // M_PAIR):
                    ostage = out_pool.tile([P, M_PAIR, J], f32, tag="ostage")
                    for mo in range(M_PAIR):
                        m = mp * M_PAIR + mo
                        for n in range(N_TILES):
                            ps = psum_pool.tile([P, N_TILE], f32, tag="ps")
                            nc.tensor.matmul(
                                ps[:],
                                kxm[:, m, :],
                                kxn[:, n * N_TILE : (n + 1) * N_TILE],
                                start=True,
                                stop=True,
                            )
                            dst = ostage[:, mo, n * N_TILE : (n + 1) * N_TILE]
                            if evict % 2 == 0:
                                nc.scalar.copy(dst, ps[:])
                            else:
                                nc.vector.tensor_copy(dst, ps[:])
                            evict += 1
                    rows = M_PAIR * P
                    nc.sync.dma_start(
                        c[bi, mp * rows : (mp + 1) * rows, :].rearrange(
                            "(mo p) j -> p mo j", p=P
                        ),
                        ostage[:],
                    )

    nc.compile()
    return nc


_NC_CACHE = None


def get_nc():
    global _NC_CACHE
    if _NC_CACHE is None:
        _NC_CACHE = build_nc()
    return _NC_CACHE


def kernel(input_0: np.ndarray, input_1: np.ndarray, _trace=False) -> np.ndarray:
    input_0 = np.ascontiguousarray(np.asarray(input_0, dtype=np.float32))
    input_1 = np.ascontiguousarray(np.asarray(input_1, dtype=np.float32))
    nc = get_nc()
    in_maps = [
        {
            "a": input_0[ci * BPC : (ci + 1) * BPC],
            "b": input_1[ci * BPC : (ci + 1) * BPC],
        }
        for ci in range(N_CORES)
    ]
    res = run_bass_kernel_spmd(nc, in_maps, list(range(N_CORES)), trace=_trace)
    out = np.concatenate([res.results[ci]["c"] for ci in range(N_CORES)], axis=0)
    if _trace:
        kernel.last_result = res
    return out
